# revision 1
# baseline (speedup 1.0000x reference)
"""LinearAttention kernel for Trainium2, 8 NeuronCores, data-parallel over batch.

Reference computation (per batch b, c=256 channels, n=4096 tokens):
  xn   = x / ||x||_c * g1 * 16                       (rms over channels)
  qkv  = Wqkv @ xn            (q,k,v each [512, n])
  q    = softmax_d(q) / 8     (softmax over dim d=64 within each of 8 heads)
  k    = softmax_n(k)         (softmax over tokens)
  ctx_h = k_h @ v_h^T         ([64, 64] per head)
  out  = Wout @ concat_h(ctx_h^T @ q_h) + bout
  out  = out / ||out||_c * g2 * 16

Sharding: 16 batches -> 8 cores x 2 batches. No collectives needed.

Layout strategy per batch:
  x, xn      [c=2x128, n]     channels on partitions
  q          [512=4x128, n]   from matmul(lhsT=WqkvT_slice, rhs=xn)
  kT, vT     [n, 512]         from matmul(lhsT=xn_slice, rhs=WqkvT_kv) - tokens on
                              partitions, so the context matmul can contract over n
  ctxT       [65, 8*64] PSUM  per head [e, d] plus row 64 = sum_n exp(k) (denominator)
  W2T        [512, 256]       (Wout_h @ ctx_h^T / kden * 0.125)^T, heads stacked
  out_pre    [2x128, n]       matmul(lhsT=W2T_block, rhs=q_sm)

Softmax-over-partitions reductions are done on the TensorE with replicated-ones /
block-diagonal-ones stationary operands (result is broadcast across partitions for
free). All matmul operands are float32r (rounded by the producing compute op),
which runs at 1 cycle/row for N>=256 on TRN2.
"""

import numpy as np

import concourse.bass as bass
import concourse.tile as tile
from concourse import bacc, mybir
from concourse.bass_utils import run_bass_kernel_spmd

F32 = mybir.dt.float32
F32R = mybir.dt.float32r

B = 16          # total batches
BL = 2          # batches per core
C = 256         # in channels
HID = 512       # heads * dim_head
HEADS = 8
DH = 64         # dim head
N = 4096        # tokens (64*64)
TN = 512        # token tile
NT = N // TN    # 8 token tiles per batch
NB = TN // 128  # 128-token blocks per tile


def build_kernel():
    nc = bacc.Bacc("TRN2", target_bir_lowering=False, debug=False, num_devices=8)

    x_d = nc.dram_tensor("x", [BL, C, N], F32, kind="ExternalInput").ap()
    wqkv_d = nc.dram_tensor("Wqkv", [3 * HID, C], F32, kind="ExternalInput").ap()
    wout_d = nc.dram_tensor("Wout", [C, HID], F32, kind="ExternalInput").ap()
    bout_d = nc.dram_tensor("bout", [C], F32, kind="ExternalInput").ap()
    g1_d = nc.dram_tensor("g1", [C], F32, kind="ExternalInput").ap()
    g2_d = nc.dram_tensor("g2", [C], F32, kind="ExternalInput").ap()
    o_d = nc.dram_tensor("out", [BL, C, N], F32, kind="ExternalOutput").ap()

    # channel-block views: channel c -> (p=c%128, cb=c//128)
    xv = x_d.rearrange("b (cb p) n -> b p cb n", cb=2)
    ov = o_d.rearrange("b (cb p) n -> b p cb n", cb=2)

    with tile.TileContext(nc) as tc:
        with (
            tc.tile_pool(name="const", bufs=1) as const,
            tc.tile_pool(name="wt", bufs=1) as wt,
            tc.tile_pool(name="stage", bufs=1) as stage,
            tc.tile_pool(name="work", bufs=3) as work,
            tc.tile_pool(name="qsm", bufs=1) as qsmp,
            tc.tile_pool(name="kvw", bufs=3) as kvw,
            tc.tile_pool(name="ps_q", bufs=3, space="PSUM") as ps_q,
            tc.tile_pool(name="ps_kv", bufs=2, space="PSUM") as ps_kv,
            tc.tile_pool(name="ps_ctx", bufs=1, space="PSUM") as ps_ctx,
        ):
            # ---------------- constants ----------------
            ones_f = const.tile([128, 128], F32)
            nc.gpsimd.memset(ones_f, 1.0)
            ones_r = const.tile([128, 128], F32R)
            nc.vector.tensor_copy(out=ones_r, in_=ones_f)

            bd_f = const.tile([128, 128], F32)
            nc.gpsimd.memset(bd_f, 0.0)
            nc.gpsimd.memset(bd_f[0:64, 0:64], 1.0)
            nc.gpsimd.memset(bd_f[64:128, 64:128], 1.0)
            bd_r = const.tile([128, 128], F32R)
            nc.vector.tensor_copy(out=bd_r, in_=bd_f)

            ident = const.tile([128, 128], F32)
            from concourse.masks import make_identity
            make_identity(nc, ident)

            scl_f = const.tile([1, 2], F32)
            nc.gpsimd.memset(scl_f, 0.125)  # attention SCALE folded into kdinv^T
            scl_r = const.tile([1, 2], F32R)
            nc.vector.tensor_copy(out=scl_r, in_=scl_f)

            ones8 = const.tile([128, 8, 2], F32)
            nc.gpsimd.memset(ones8, 1.0)

            g1c = const.tile([128, 2], F32)
            nc.sync.dma_start(out=g1c, in_=g1_d.rearrange("(cb p) -> p cb", cb=2))
            g2c = const.tile([128, 2], F32)
            nc.sync.dma_start(out=g2c, in_=g2_d.rearrange("(cb p) -> p cb", cb=2))
            boutc = const.tile([128, 2], F32)
            nc.sync.dma_start(out=boutc, in_=bout_d.rearrange("(cb p) -> p cb", cb=2))

            # ---------------- weights: load + transpose on PE ----------------
            # Wqkv [1536, 256] -> WqkvT [c(2x128 part), 1536], g1 folded into rows
            wq_nat = stage.tile([128, 12, 256], F32, tag="wnat")
            nc.sync.dma_start(
                out=wq_nat, in_=wqkv_d.rearrange("(ob p) c -> p ob c", p=128)
            )
            wqkvT = wt.tile([128, 2, 1536], F32R)
            for ob in range(12):
                for cb in range(2):
                    pt = ps_q.tile([128, 512], F32, tag="q")
                    nc.tensor.transpose(
                        pt[:, 0:128], wq_nat[:, ob, cb * 128:(cb + 1) * 128], ident
                    )
                    nc.vector.tensor_scalar_mul(
                        out=wqkvT[:, cb, ob * 128:(ob + 1) * 128],
                        in0=pt[:, 0:128],
                        scalar1=g1c[:, cb:cb + 1],
                    )
            # Wout [256, 512] -> WoutT [hid(4x128 part), 256]
            wo_nat = stage.tile([128, 2, 512], F32, tag="wnat")
            nc.sync.dma_start(
                out=wo_nat, in_=wout_d.rearrange("(ob p) h -> p ob h", p=128)
            )
            # per-head layout [e=64 (base 0), h, o=256] so K=64 matmuls align
            woutT = wt.tile([64, 8, 256], F32R)
            for h in range(HEADS):
                for ob in range(2):
                    pt = ps_q.tile([128, 512], F32, tag="q")
                    nc.tensor.transpose(
                        pt[0:64, 0:128], wo_nat[:, ob, h * 64:(h + 1) * 64], ident
                    )
                    nc.vector.tensor_copy(
                        out=woutT[:, h, ob * 128:(ob + 1) * 128], in_=pt[0:64, 0:128]
                    )

            # ---------------- per-batch pipeline ----------------
            for bl in range(BL):
                ctx = ps_ctx.tile([128, 512], F32, tag="ctx")  # [66, 8*64] used
                nc.vector.memset(ctx, 0.0)  # clear data + has_written bits
                q_sm = qsmp.tile([128, 4, N], F32R, tag="qsm")

                for j in range(NT):
                    t0 = j * TN
                    # -- load x tile, rms-normalize over channels --
                    xt = work.tile([128, 2, TN], F32, tag="xt", bufs=2)
                    nc.sync.dma_start(out=xt, in_=xv[bl, :, :, t0:t0 + TN])
                    x2 = work.tile([128, 2, TN], F32R, tag="x2", bufs=2)
                    nc.scalar.activation(
                        out=x2, in_=xt, func=mybir.ActivationFunctionType.Square
                    )
                    ssq = ps_q.tile([128, 512], F32, tag="q")
                    for cb in range(2):
                        nc.tensor.matmul(
                            ssq, ones_r, x2[:, cb, :], start=(cb == 0), stop=(cb == 1)
                        )
                    # s = sqrt(ssq/256); sinv = 1/s = 16/||x||  (replicated [128,TN])
                    s_sb = work.tile([128, TN], F32, tag="s", bufs=2)
                    nc.scalar.activation(
                        out=s_sb, in_=ssq,
                        func=mybir.ActivationFunctionType.Sqrt, scale=1.0 / 256.0,
                    )
                    sinv = work.tile([128, TN], F32, tag="sinv", bufs=2)
                    nc.vector.reciprocal(out=sinv, in_=s_sb)
                    xn = work.tile([128, 2, TN], F32R, tag="xn", bufs=2)
                    for cb in range(2):
                        nc.vector.tensor_mul(xn[:, cb, :], xt[:, cb, :], sinv)

                    # -- q = Wq @ xn  -> [4x128, TN], then softmax over d --
                    expq = work.tile([128, 4, TN], F32R, tag="expq", bufs=2)
                    for ob in range(4):
                        pq = ps_q.tile([128, 512], F32, tag="q")
                        for cb in range(2):
                            nc.tensor.matmul(
                                pq,
                                wqkvT[:, cb, ob * 128:(ob + 1) * 128],
                                xn[:, cb, :],
                                start=(cb == 0), stop=(cb == 1),
                            )
                        nc.scalar.activation(
                            out=expq[:, ob, :], in_=pq,
                            func=mybir.ActivationFunctionType.Exp,
                        )
                    # per-head sums over d (block-diag ones), replicated; recip; mult
                    for ob in range(4):
                        pd = ps_q.tile([128, 512], F32, tag="q")
                        nc.tensor.matmul(pd, bd_r, expq[:, ob, :], start=True, stop=True)
                        qdi = work.tile([128, TN], F32, tag="qdi", bufs=2)
                        nc.vector.reciprocal(out=qdi, in_=pd)
                        nc.vector.tensor_mul(
                            q_sm[:, ob, t0:t0 + TN], expq[:, ob, :], qdi
                        )

                    # -- kT/vT = (xn_block)^T @ Wkv -> [128 tok, 1024]; exp(k); ctx --
                    for nb in range(NB):
                        pkv = ps_kv.tile([128, 1024], F32, tag="kv")
                        xn_blk_lo = xn[:, 0, nb * 128:(nb + 1) * 128]
                        xn_blk_hi = xn[:, 1, nb * 128:(nb + 1) * 128]
                        for half in range(2):
                            nc.tensor.matmul(
                                pkv[:, half * 512:(half + 1) * 512],
                                xn_blk_lo,
                                wqkvT[:, 0, 512 + half * 512:1024 + half * 512],
                                start=True, stop=False,
                            )
                            nc.tensor.matmul(
                                pkv[:, half * 512:(half + 1) * 512],
                                xn_blk_hi,
                                wqkvT[:, 1, 512 + half * 512:1024 + half * 512],
                                start=False, stop=True,
                            )
                        expk = kvw.tile([128, 8, 64], F32R, tag="expk")
                        nc.scalar.activation(
                            out=expk, in_=pkv[:, 0:512],
                            func=mybir.ActivationFunctionType.Exp,
                        )
                        vt = kvw.tile([128, 8, 66], F32R, tag="vt")
                        vsrc = pkv[:, 512:1024].rearrange("p (h e) -> p h e", h=8)
                        if nb % 2 == 0:
                            nc.vector.tensor_copy(out=vt[:, :, 0:64], in_=vsrc)
                        else:
                            nc.scalar.activation(
                                out=vt[:, :, 0:64], in_=vsrc,
                                func=mybir.ActivationFunctionType.Copy,
                            )
                        nc.vector.tensor_copy(out=vt[:, :, 64:66], in_=ones8)
                        # ctx[e(+den), d] += vt_ext^T @ expk, per head
                        gnb = j * NB + nb
                        for h in range(HEADS):
                            nc.tensor.matmul(
                                ctx[0:66, h * 64:(h + 1) * 64],
                                vt[:, h, :],
                                expk[:, h, :],
                                start=False, stop=(gnb == N // 128 - 1),
                                skip_group_check=True,
                            )

                # -- batch epilogue: W2T = (Wout_h @ ctxT_h / kden * 0.125)^T --
                kdinv = work.tile([1, 512], F32R, tag="kdi", bufs=1)
                with nc.allow_low_precision(reason="fp32r lhsT for kden transpose"):
                    nc.vector.reciprocal(out=kdinv, in_=ctx[64:65, :])
                ctx_sb = work.tile([64, 512], F32R, tag="ctxsb", bufs=1)
                nc.vector.tensor_copy(out=ctx_sb, in_=ctx[0:64, :])
                # transpose kdinv rows -> columns [64, 8] (x0.125 via rhs)
                pkd = ps_q.tile([128, 512], F32, tag="q")
                for h in range(HEADS):
                    nc.tensor.matmul(
                        pkd[0:64, 2 * h:2 * h + 2],
                        kdinv[0:1, h * 64:(h + 1) * 64],
                        scl_r,
                        start=True, stop=True,
                    )
                kdcol = work.tile([64, 8, 1], F32, tag="kdcol", bufs=1)
                pkd_v = pkd[0:64, 0:16].rearrange("p (h t) -> p h t", t=2)
                nc.vector.tensor_copy(out=kdcol, in_=pkd_v[:, :, 0:1])
                w2stg = stage.tile([64, 8, 256], F32R, tag="wnat")
                for h in range(HEADS):
                    pw2 = ps_q.tile([128, 512], F32, tag="q")
                    nc.tensor.matmul(
                        pw2[0:64, 0:256],
                        ctx_sb[:, h * 64:(h + 1) * 64],
                        woutT[:, h, :],
                        start=True, stop=True,
                    )
                    nc.vector.tensor_scalar_mul(
                        out=w2stg[:, h, :],
                        in0=pw2[0:64, 0:256],
                        scalar1=kdcol[:, h, :],
                    )
                w2T = stage.tile([128, 4, 256], F32R, tag="w2T")
                for h in range(HEADS):
                    nc.sync.dma_start(
                        out=w2T[(h % 2) * 64:(h % 2) * 64 + 64, h // 2, :],
                        in_=w2stg[:, h, :],
                    )

                # -- stage B: out_pre = W2T^T @ q_sm + bout; rms-norm; store --
                for j in range(NT):
                    t0 = j * TN
                    y = work.tile([128, 2, TN], F32, tag="y", bufs=2)
                    for ob in range(2):
                        po = ps_q.tile([128, 512], F32, tag="q")
                        for kb in range(4):
                            nc.tensor.matmul(
                                po,
                                w2T[:, kb, ob * 128:(ob + 1) * 128],
                                q_sm[:, kb, t0:t0 + TN],
                                start=(kb == 0), stop=(kb == 3),
                            )
                        nc.vector.tensor_scalar_add(
                            out=y[:, ob, :], in0=po, scalar1=boutc[:, ob:ob + 1]
                        )
                    y2 = work.tile([128, 2, TN], F32R, tag="y2", bufs=2)
                    nc.scalar.activation(
                        out=y2, in_=y, func=mybir.ActivationFunctionType.Square
                    )
                    ssqo = ps_q.tile([128, 512], F32, tag="q")
                    for cb in range(2):
                        nc.tensor.matmul(
                            ssqo, ones_r, y2[:, cb, :], start=(cb == 0), stop=(cb == 1)
                        )
                    so = work.tile([128, TN], F32, tag="so", bufs=2)
                    nc.scalar.activation(
                        out=so, in_=ssqo,
                        func=mybir.ActivationFunctionType.Sqrt, scale=1.0 / 256.0,
                    )
                    rinv = work.tile([128, TN], F32, tag="rinv", bufs=2)
                    nc.vector.reciprocal(out=rinv, in_=so)
                    for ob in range(2):
                        tmp = work.tile([128, TN], F32, tag="tmp", bufs=2)
                        nc.vector.tensor_mul(tmp, y[:, ob, :], rinv)
                        nc.scalar.activation(
                            out=y[:, ob, :], in_=tmp,
                            func=mybir.ActivationFunctionType.Identity,
                            scale=g2c[:, ob:ob + 1],
                        )
                    nc.sync.dma_start(out=ov[bl, :, :, t0:t0 + TN], in_=y)

    nc.finalize()
    return nc


_NC_CACHE = None


def kernel(x, g1, Wqkv, Wout, bout, g2):
    global _NC_CACHE
    x = np.ascontiguousarray(np.asarray(x, dtype=np.float32))
    g1 = np.asarray(g1, dtype=np.float32)
    Wqkv = np.ascontiguousarray(np.asarray(Wqkv, dtype=np.float32))
    Wout = np.ascontiguousarray(np.asarray(Wout, dtype=np.float32))
    bout = np.asarray(bout, dtype=np.float32)
    g2 = np.asarray(g2, dtype=np.float32)

    b, c, H, W = x.shape
    xr = x.reshape(b, c, H * W)
    if _NC_CACHE is None:
        _NC_CACHE = build_kernel()
    nc = _NC_CACHE

    in_maps = []
    for core in range(8):
        in_maps.append({
            "x": np.ascontiguousarray(xr[core * BL:(core + 1) * BL]),
            "Wqkv": Wqkv, "Wout": Wout, "bout": bout, "g1": g1, "g2": g2,
        })
    res = run_bass_kernel_spmd(nc, in_maps, core_ids=list(range(8)))
    out = np.concatenate([m["out"] for m in res.results], axis=0)
    return out.reshape(b, c, H, W).astype(np.float32)


if __name__ == "__main__":
    rng = np.random.default_rng(0)
    x = rng.standard_normal((16, 256, 64, 64), dtype=np.float32)
    inputs = dict(
        x=x,
        g1=np.ones(256, np.float32),
        Wqkv=(rng.standard_normal((1536, 256), dtype=np.float32) * 256 ** -0.5),
        Wout=(rng.standard_normal((256, 512), dtype=np.float32) * 512 ** -0.5),
        bout=np.zeros(256, np.float32),
        g2=np.ones(256, np.float32),
    )
    out = kernel(**inputs)
    print("out", out.shape, out.dtype, np.abs(out).max())

